# revision 4
# baseline (speedup 1.0000x reference)
"""DeepAR LSTM forward on 8 Trainium2 NeuronCores.

Strategy (data-parallel over batch):
  - B=1024 split as 128 batch elements per core; the L=512 time recurrence
    runs locally on each core.
  - Cell state is kept batch-major on chip: batch on the 128 partitions,
    gates/hidden units on the free dim.  Every elementwise operand pair then
    shares partition base 0, which the walrus verifier requires for
    SBUF-SBUF TensorTensor ops.
  - Gate matmuls compute gates[b, gate] = lhsT.T @ rhs with the per-step
    DATA as the stationary operand (lhsT = [x_t; 1] or [v; o] feature-major)
    and the WEIGHTS as the moving operand.  Biases ride a ones-row in lhsT.
  - tanh is folded into sigmoid: tanh(z) = 2*sigmoid(2z) - 1 (g-gate weight
    columns pre-scaled by 2), so one sigmoid covers i, f, w=sigma(2g).
  - h = o*tanh(c) is never materialized: with v = sigmoid(2c)*o we have
    h = 2v - o, and (2, -1) is folded into the recurrent/head weights.
    Per step a PE transpose turns [128, 32] (v|o) into the feature-major
    [32, 128] stationary operand for the next step.
  - Host does all layout transposes in numpy (free for HW time).
"""

import sys
from contextlib import ExitStack

import numpy as np

sys.path.insert(0, "/opt/trn_rl_repo")

import concourse.bacc as bacc  # noqa: E402
import concourse.bass as bass  # noqa: E402
import concourse.mybir as mybir  # noqa: E402
import concourse.tile as tile  # noqa: E402
from concourse.masks import make_identity  # noqa: E402

F32 = mybir.dt.float32
AF = mybir.ActivationFunctionType
ALU = mybir.AluOpType

L, B, IN, H, OBS = 512, 1024, 32, 16, 16
NCORES = 8
B_LOC = B // NCORES          # 128 batch rows per core
T_C = 32                     # timesteps per SBUF chunk
HEAD_G = 4                   # timesteps per head PSUM flush

# gate column order in the [*, 64] gate tensors
GI, GF, GW, GO = 0, 16, 32, 48


def build_nc(steps: int = L, b_loc: int = B_LOC):
    """Emit the per-core Bass program (identical on all cores)."""
    nc = bacc.Bacc(None, target_bir_lowering=False)

    xT = nc.dram_tensor("xT", [IN + 1, steps * b_loc], F32, kind="ExternalInput")
    w_x = nc.dram_tensor("w_x", [IN + 1, 64], F32, kind="ExternalInput")
    w_vo = nc.dram_tensor("w_vo", [32, 64], F32, kind="ExternalInput")
    w_h = nc.dram_tensor("w_h", [33, 2 * OBS], F32, kind="ExternalInput")

    outB = nc.dram_tensor("outB", [b_loc, steps * 2 * OBS], F32,
                          kind="ExternalOutput")
    h_last = nc.dram_tensor("h_last", [b_loc, H], F32, kind="ExternalOutput")
    c_last = nc.dram_tensor("c_last", [b_loc, H], F32, kind="ExternalOutput")

    n_chunks = (steps + T_C - 1) // T_C

    with tile.TileContext(nc) as tc, ExitStack() as ctx:
        singles = ctx.enter_context(tc.tile_pool(name="singles", bufs=1))
        xpool = ctx.enter_context(tc.tile_pool(name="xpool", bufs=2))
        vtpool = ctx.enter_context(tc.tile_pool(name="vtpool", bufs=2))
        opool = ctx.enter_context(tc.tile_pool(name="opool", bufs=2))
        spool = ctx.enter_context(tc.tile_pool(name="spool", bufs=3))
        small = ctx.enter_context(tc.tile_pool(name="small", bufs=3))
        vobp = ctx.enter_context(tc.tile_pool(name="vobp", bufs=3))
        psum_g = ctx.enter_context(tc.tile_pool(name="psum_g", bufs=4, space="PSUM"))
        psum_t = ctx.enter_context(tc.tile_pool(name="psum_t", bufs=2, space="PSUM"))
        psum_h = ctx.enter_context(tc.tile_pool(name="psum_h", bufs=2, space="PSUM"))

        # --- constants ---
        wx = singles.tile([IN + 1, 64], F32)
        wvo = singles.tile([32, 64], F32)
        wh = singles.tile([33, 2 * OBS], F32)
        ident = singles.tile([128, 128], F32)
        nc.sync.dma_start(out=wx, in_=w_x[:])
        nc.sync.dma_start(out=wvo, in_=w_vo[:])
        nc.sync.dma_start(out=wh, in_=w_h[:])
        make_identity(nc, ident)

        # persistent state
        cA = singles.tile([b_loc, H], F32)
        cB = singles.tile([b_loc, H], F32)
        voT_init = singles.tile([32, b_loc], F32)
        nc.vector.memset(cA, 0.0)
        nc.vector.memset(voT_init, 0.0)
        c_tiles = (cA, cB)

        voT_prev_ap = voT_init[:]        # [32, b_loc] feature-major (v; o)
        last_vo_b = None

        for ck in range(n_chunks):
            t0 = ck * T_C
            t1 = min(t0 + T_C, steps)
            span = t1 - t0

            xc = xpool.tile([IN + 1, T_C * b_loc], F32)
            nc.sync.dma_start(
                out=xc[:, : span * b_loc],
                in_=xT[:, t0 * b_loc : t1 * b_loc],
            )
            voT = vtpool.tile([33, T_C * b_loc], F32)
            # ones row for the head-bias trick
            nc.gpsimd.memset(voT[32:33, :], 1.0)
            oc = opool.tile([b_loc, T_C * 2 * OBS], F32)

            for t in range(t0, t1):
                tl = t - t0
                sl = slice(tl * b_loc, (tl + 1) * b_loc)

                pg = psum_g.tile([b_loc, 64], F32)
                nc.tensor.matmul(pg, xc[:, sl], wx, start=True, stop=False)
                nc.tensor.matmul(pg, voT_prev_ap, wvo, start=False, stop=True)

                s = spool.tile([b_loc, 48], F32)
                vo_b = vobp.tile([b_loc, 32], F32)   # v cols 0:16, o cols 16:32
                nc.scalar.activation(out=s, in_=pg[:, 0:48], func=AF.Sigmoid)
                nc.scalar.activation(
                    out=vo_b[:, 16:32], in_=pg[:, 48:64], func=AF.Sigmoid
                )

                c_prev = c_tiles[t % 2]
                c_new = c_tiles[(t + 1) % 2]
                wt = small.tile([b_loc, H], F32)
                t1m = small.tile([b_loc, H], F32)
                m1 = small.tile([b_loc, H], F32)
                nc.vector.tensor_scalar(
                    out=wt, in0=s[:, GW : GW + H], scalar1=2.0, scalar2=-1.0,
                    op0=ALU.mult, op1=ALU.add,
                )
                nc.vector.tensor_tensor(t1m, s[:, GI : GI + H], wt, op=ALU.mult)
                nc.vector.tensor_tensor(m1, s[:, GF : GF + H], c_prev, op=ALU.mult)
                nc.vector.tensor_tensor(c_new, m1, t1m, op=ALU.add)

                u = small.tile([b_loc, H], F32)
                nc.scalar.activation(out=u, in_=c_new, func=AF.Sigmoid, scale=2.0)
                nc.vector.tensor_tensor(
                    vo_b[:, 0:16], u, vo_b[:, 16:32], op=ALU.mult
                )

                # feature-major (v; o) for the next step + heads
                pt = psum_t.tile([32, b_loc], F32)
                nc.tensor.transpose(pt[:], vo_b[:], ident[:])
                nc.vector.tensor_copy(voT[0:32, sl], pt[:])
                voT_prev_ap = voT[0:32, sl]
                last_vo_b = vo_b

                # heads: one matmul per step into a 4-step PSUM tile
                gl = tl % HEAD_G
                if gl == 0:
                    ph = psum_h.tile([b_loc, HEAD_G * 2 * OBS], F32)
                hsl = slice(gl * 2 * OBS, (gl + 1) * 2 * OBS)
                nc.tensor.matmul(
                    ph[:, hsl], voT[0:33, sl], wh, start=True, stop=True
                )
                if gl == HEAD_G - 1 or t + 1 == t1:
                    g0 = tl - gl
                    osl = slice(g0 * 2 * OBS, (tl + 1) * 2 * OBS)
                    n = (gl + 1) * 2 * OBS
                    nc.scalar.activation(
                        out=oc[:, osl], in_=ph[:, :n], func=AF.Identity
                    )

            nc.sync.dma_start(
                out=outB[:, t0 * 2 * OBS : t1 * 2 * OBS],
                in_=oc[:, : span * 2 * OBS],
            )

        # h = 2v - o ; c
        hl = small.tile([b_loc, H], F32)
        nc.vector.scalar_tensor_tensor(
            out=hl, in0=last_vo_b[:, 0:16], scalar=2.0, in1=last_vo_b[:, 16:32],
            op0=ALU.mult, op1=ALU.subtract,
        )
        nc.sync.dma_start(out=h_last[:], in_=hl[:])
        nc.sync.dma_start(out=c_last[:], in_=c_tiles[steps % 2][:])

    return nc


def prep_weights(W_ih, W_hh, b_ih, b_hh, W_mu, b_mu, W_sig, b_sig):
    W = np.asarray(W_ih, np.float32)        # (64, 32)  rows i,f,g,o
    U = np.asarray(W_hh, np.float32)        # (64, 16)
    b = np.asarray(b_ih, np.float32) + np.asarray(b_hh, np.float32)
    Wm = np.asarray(W_mu, np.float32)       # (16, 16)
    Ws = np.asarray(W_sig, np.float32)

    # gate columns [i, f, w(g), o]; g block pre-scaled by 2 for the tanh fold
    w_x = np.zeros((IN + 1, 64), np.float32)
    w_vo = np.zeros((32, 64), np.float32)
    for bi in range(4):
        r = slice(bi * H, (bi + 1) * H)
        col = slice(bi * H, (bi + 1) * H)
        sc = 2.0 if bi == 2 else 1.0
        w_x[:IN, col] = sc * W[r].T
        w_x[IN, col] = sc * b[r]
        w_vo[0:16, col] = sc * 2.0 * U[r].T
        w_vo[16:32, col] = sc * -1.0 * U[r].T

    w_h = np.zeros((33, 2 * OBS), np.float32)
    w_h[0:16, :OBS] = 2.0 * Wm.T
    w_h[0:16, OBS:] = 2.0 * Ws.T
    w_h[16:32, :OBS] = -Wm.T
    w_h[16:32, OBS:] = -Ws.T
    w_h[32, :OBS] = np.asarray(b_mu, np.float32)
    w_h[32, OBS:] = np.asarray(b_sig, np.float32)
    return w_x, w_vo, w_h


def kernel(external_input_seq, W_ih, W_hh, b_ih, b_hh, W_mu, b_mu, W_sig, b_sig,
           _trace=False):
    from concourse.bass_utils import run_bass_kernel_spmd

    x = np.asarray(external_input_seq, np.float32)      # (L, B, IN)
    w_x, w_vo, w_h = prep_weights(
        W_ih, W_hh, b_ih, b_hh, W_mu, b_mu, W_sig, b_sig
    )

    nc = build_nc(L, B_LOC)
    nc.compile()
    in_maps = []
    for c in range(NCORES):
        xc = x[:, c * B_LOC : (c + 1) * B_LOC, :]       # (L, B_loc, IN)
        xT = np.empty((IN + 1, L * B_LOC), np.float32)
        xT[:IN] = xc.transpose(2, 0, 1).reshape(IN, L * B_LOC)
        xT[IN] = 1.0
        in_maps.append({"xT": xT, "w_x": w_x, "w_vo": w_vo, "w_h": w_h})

    res = run_bass_kernel_spmd(nc, in_maps, list(range(NCORES)), trace=_trace)

    mu = np.empty((L, B, OBS), np.float32)
    sg = np.empty((L, B, OBS), np.float32)
    hl = np.empty((1, B, H), np.float32)
    cl = np.empty((1, B, H), np.float32)
    for c in range(NCORES):
        r = res.results[c]
        bs = slice(c * B_LOC, (c + 1) * B_LOC)
        ot = r["outB"].reshape(B_LOC, L, 2 * OBS).transpose(1, 0, 2)
        mu[:, bs, :] = ot[:, :, :OBS]
        sg[:, bs, :] = ot[:, :, OBS:]
        hl[0, bs, :] = r["h_last"]
        cl[0, bs, :] = r["c_last"]

    if _trace:
        kernel.last_exec_time_ns = res.exec_time_ns
    return mu, sg, hl, cl


# revision 7
# speedup vs baseline: 1.1161x; 1.1161x over previous
"""DeepAR LSTM forward on 8 Trainium2 NeuronCores.

Strategy (data-parallel over batch):
  - B=1024 split as 128 batch elements per core; the L=512 time recurrence
    runs locally on each core.
  - Cell state is batch-major on chip (batch on partitions, gates on the
    free dim) so every elementwise operand pair shares partition base 0,
    which the walrus verifier requires for SBUF-SBUF TensorTensor ops.
  - ONE matmul per step computes all gate pre-activations: the stationary
    operand is a combined [96, 128] column of a chunk tile holding
    [x_t (32); 1; zeros; v_{t-1}; o_{t-1}], the moving operand is a
    combined weight matrix.  The bias rides the ones-row.  float32r makes
    the matmul single-pass (vs fp32's LOW+HIGH double pass).
  - tanh is folded into sigmoid: tanh(z) = 2*sigmoid(2z) - 1 (g columns
    pre-scaled by 2), so one sigmoid covers i, f, w=sigma(2g).
  - h = o*tanh(c) is never materialized: with v = sigmoid(2c)*o we have
    h = 2v - o, folded into the recurrent/head weights.  A per-step PE
    transpose turns [128, 32] (v|o) into feature-major [32, 128], written
    into the NEXT step's stationary slot.
  - Heads are weight-stationary bulk matmuls over 4 steps of (v, o)
    feature-major slots; outputs stay feature-major, host transposes.
"""

import sys
from contextlib import ExitStack

import numpy as np

sys.path.insert(0, "/opt/trn_rl_repo")

import concourse.bacc as bacc  # noqa: E402
import concourse.bass as bass  # noqa: E402
import concourse.mybir as mybir  # noqa: E402
import concourse.tile as tile  # noqa: E402
from concourse.masks import make_identity  # noqa: E402

F32 = mybir.dt.float32
F32R = mybir.dt.float32r
AF = mybir.ActivationFunctionType
ALU = mybir.AluOpType

L, B, IN, H, OBS = 512, 1024, 32, 16, 16
NCORES = 8
B_LOC = B // NCORES          # 128 batch rows per core
T_C = 32                     # timesteps per SBUF chunk
HEAD_G = 4                   # timesteps per head matmul

# gate column order in the [*, 64] gate tensors
GI, GF, GW, GO = 0, 16, 32, 48
# combined stationary rows: x 0:32, ones 32, zeros 33:64, v 64:80, o 80:96
RV, RO, R1 = 64, 80, 96


def build_nc(steps: int = L, b_loc: int = B_LOC):
    """Emit the per-core Bass program (identical on all cores)."""
    nc = bacc.Bacc(None, target_bir_lowering=False)

    xT = nc.dram_tensor("xT", [IN + 1, steps * b_loc], F32R, kind="ExternalInput")
    w_all = nc.dram_tensor("w_all", [R1, 64], F32R, kind="ExternalInput")
    w_h = nc.dram_tensor("w_h", [32, 2 * OBS], F32R, kind="ExternalInput")
    b_h = nc.dram_tensor("b_h", [2 * OBS, 1], F32, kind="ExternalInput")

    outT = nc.dram_tensor("outT", [2 * OBS, steps * b_loc], F32,
                          kind="ExternalOutput")
    h_last = nc.dram_tensor("h_last", [b_loc, H], F32, kind="ExternalOutput")
    c_last = nc.dram_tensor("c_last", [b_loc, H], F32, kind="ExternalOutput")

    n_chunks = (steps + T_C - 1) // T_C
    assert steps % T_C == 0 and T_C % HEAD_G == 0

    with tile.TileContext(nc) as tc, ExitStack() as ctx:
        singles = ctx.enter_context(tc.tile_pool(name="singles", bufs=1))
        xpool = ctx.enter_context(tc.tile_pool(name="xpool", bufs=3))
        opool = ctx.enter_context(tc.tile_pool(name="opool", bufs=2))
        spool = ctx.enter_context(tc.tile_pool(name="spool", bufs=3))
        small = ctx.enter_context(tc.tile_pool(name="small", bufs=3))
        vobp = ctx.enter_context(tc.tile_pool(name="vobp", bufs=3))
        psum_g = ctx.enter_context(tc.tile_pool(name="psum_g", bufs=4, space="PSUM"))
        psum_t = ctx.enter_context(tc.tile_pool(name="psum_t", bufs=2, space="PSUM"))
        psum_h = ctx.enter_context(tc.tile_pool(name="psum_h", bufs=2, space="PSUM"))

        # --- constants ---
        wall = singles.tile([R1, 64], F32R)
        # head weights live at partitions 64:96 so the head matmul's two
        # operands share base partition 64 (hardware requirement)
        wh_t = singles.tile([R1, 2 * OBS], F32R)
        wh = wh_t[RV:R1, :]
        bh = singles.tile([2 * OBS, 1], F32)
        ident = singles.tile([128, 128], F32)
        nc.sync.dma_start(out=wall, in_=w_all[:])
        nc.sync.dma_start(out=wh, in_=w_h[:])
        nc.sync.dma_start(out=bh, in_=b_h[:])
        make_identity(nc, ident)

        cA = singles.tile([b_loc, H], F32)
        cB = singles.tile([b_loc, H], F32)
        nc.vector.memset(cA, 0.0)
        c_tiles = (cA, cB)

        # trailing stationary slot for step L-1's (v, o) -> step-511 heads
        xlast = singles.tile([R1, b_loc], F32R)

        def new_chunk(ck):
            """Allocate chunk ck's stationary tile, queue DMA + memsets."""
            t0 = ck * T_C
            xc = xpool.tile([R1, T_C * b_loc], F32R)
            # zeros band 33:64 (memset 32:64 first; DMA then rewrites row 32)
            nc.gpsimd.memset(xc[32:64, :].bitcast(F32), 0.0)
            nc.sync.dma_start(
                out=xc[: IN + 1, :],
                in_=xT[:, t0 * b_loc : (t0 + T_C) * b_loc],
            )
            if ck == 0:
                nc.gpsimd.memset(xc[RV:R1, :b_loc].bitcast(F32), 0.0)
            return xc

        xc_cur = new_chunk(0)
        xc_next = new_chunk(1) if n_chunks > 1 else None
        last_vo_b = None

        for ck in range(n_chunks):
            t0 = ck * T_C
            oc = opool.tile([2 * OBS, T_C * b_loc], F32)

            for tl in range(T_C):
                t = t0 + tl
                sl = slice(tl * b_loc, (tl + 1) * b_loc)

                pg = psum_g.tile([b_loc, 64], F32)
                nc.tensor.matmul(pg, xc_cur[0:R1, sl], wall,
                                 start=True, stop=True)

                s = spool.tile([b_loc, 48], F32)
                vo_b = vobp.tile([b_loc, 32], F32)   # v 0:16, o 16:32
                nc.scalar.activation(out=s, in_=pg[:, 0:48], func=AF.Sigmoid)
                nc.scalar.activation(
                    out=vo_b[:, 16:32], in_=pg[:, 48:64], func=AF.Sigmoid
                )

                c_prev = c_tiles[t % 2]
                c_new = c_tiles[(t + 1) % 2]
                wt = small.tile([b_loc, H], F32)
                t1m = small.tile([b_loc, H], F32)
                m1 = small.tile([b_loc, H], F32)
                nc.vector.tensor_scalar(
                    out=wt, in0=s[:, GW : GW + H], scalar1=2.0, scalar2=-1.0,
                    op0=ALU.mult, op1=ALU.add,
                )
                # f*c_prev on GpSimd, off the DVE chain
                nc.gpsimd.tensor_tensor(m1, s[:, GF : GF + H], c_prev,
                                        op=ALU.mult)
                nc.vector.tensor_tensor(t1m, s[:, GI : GI + H], wt, op=ALU.mult)
                nc.vector.tensor_tensor(c_new, m1, t1m, op=ALU.add)

                u = small.tile([b_loc, H], F32)
                nc.scalar.activation(out=u, in_=c_new, func=AF.Sigmoid, scale=2.0)
                nc.vector.tensor_tensor(
                    vo_b[:, 0:16], u, vo_b[:, 16:32], op=ALU.mult
                )

                # (v; o) feature-major into the NEXT stationary slot
                pt = psum_t.tile([32, b_loc], F32)
                nc.tensor.transpose(pt[:], vo_b[:], ident[:])
                if tl + 1 < T_C:
                    dst = xc_cur[RV:R1, (tl + 1) * b_loc : (tl + 2) * b_loc]
                elif xc_next is not None:
                    dst = xc_next[RV:R1, 0:b_loc]
                else:
                    dst = xlast[RV:R1, :]
                nc.vector.tensor_copy(dst, pt[:])
                last_vo_b = vo_b

                # heads over completed 4-slot windows [4k, 4k+4)
                if tl % HEAD_G == HEAD_G - 2:
                    w0 = tl + 2 - HEAD_G
                    wsl = slice(w0 * b_loc, (w0 + HEAD_G) * b_loc)
                    ph = psum_h.tile([2 * OBS, HEAD_G * b_loc], F32)
                    nc.tensor.matmul(ph, wh, xc_cur[RV:R1, wsl],
                                     start=True, stop=True)
                    nc.scalar.activation(out=oc[:, wsl], in_=ph[:],
                                         func=AF.Identity, bias=bh[:])

            # chunk ck's slots s hold (v,o)_{t0+s-1} -> output steps t0-1..t0+30
            if ck == 0:
                nc.sync.dma_start(
                    out=outT[:, 0 : (T_C - 1) * b_loc],
                    in_=oc[:, b_loc:],
                )
            else:
                nc.sync.dma_start(
                    out=outT[:, (t0 - 1) * b_loc : (t0 + T_C - 1) * b_loc],
                    in_=oc[:],
                )
            xc_cur = xc_next
            xc_next = new_chunk(ck + 2) if ck + 2 < n_chunks else None

        # final step's heads from xlast
        ph = psum_h.tile([2 * OBS, b_loc], F32)
        ol = small.tile([2 * OBS, b_loc], F32)
        nc.tensor.matmul(ph, wh, xlast[RV:R1, :], start=True, stop=True)
        nc.scalar.activation(out=ol, in_=ph[:], func=AF.Identity, bias=bh[:])
        nc.sync.dma_start(out=outT[:, (steps - 1) * b_loc :], in_=ol[:])

        # h = 2v - o ; c
        hl = small.tile([b_loc, H], F32)
        nc.vector.scalar_tensor_tensor(
            out=hl, in0=last_vo_b[:, 0:16], scalar=2.0, in1=last_vo_b[:, 16:32],
            op0=ALU.mult, op1=ALU.subtract,
        )
        nc.sync.dma_start(out=h_last[:], in_=hl[:])
        nc.sync.dma_start(out=c_last[:], in_=c_tiles[steps % 2][:])

    return nc


def prep_weights(W_ih, W_hh, b_ih, b_hh, W_mu, b_mu, W_sig, b_sig):
    W = np.asarray(W_ih, np.float32)        # (64, 32)  rows i,f,g,o
    U = np.asarray(W_hh, np.float32)        # (64, 16)
    b = np.asarray(b_ih, np.float32) + np.asarray(b_hh, np.float32)
    Wm = np.asarray(W_mu, np.float32)       # (16, 16)
    Ws = np.asarray(W_sig, np.float32)

    # gate columns [i, f, w(g), o]; g block pre-scaled by 2 for the tanh fold
    w_all = np.zeros((R1, 64), np.float32)
    for bi in range(4):
        r = slice(bi * H, (bi + 1) * H)
        col = slice(bi * H, (bi + 1) * H)
        sc = 2.0 if bi == 2 else 1.0
        w_all[:IN, col] = sc * W[r].T
        w_all[IN, col] = sc * b[r]
        w_all[RV : RV + H, col] = sc * 2.0 * U[r].T
        w_all[RO : RO + H, col] = sc * -1.0 * U[r].T

    w_h = np.zeros((32, 2 * OBS), np.float32)
    w_h[0:16, :OBS] = 2.0 * Wm.T
    w_h[0:16, OBS:] = 2.0 * Ws.T
    w_h[16:32, :OBS] = -Wm.T
    w_h[16:32, OBS:] = -Ws.T
    b_h = np.concatenate([np.asarray(b_mu, np.float32),
                          np.asarray(b_sig, np.float32)]).reshape(2 * OBS, 1)
    return w_all, w_h, b_h


def kernel(external_input_seq, W_ih, W_hh, b_ih, b_hh, W_mu, b_mu, W_sig, b_sig,
           _trace=False):
    from concourse.bass_utils import run_bass_kernel_spmd

    x = np.asarray(external_input_seq, np.float32)      # (L, B, IN)
    w_all, w_h, b_h = prep_weights(W_ih, W_hh, b_ih, b_hh, W_mu, b_mu,
                                   W_sig, b_sig)

    nc = build_nc(L, B_LOC)
    nc.compile()
    in_maps = []
    for c in range(NCORES):
        xc = x[:, c * B_LOC : (c + 1) * B_LOC, :]       # (L, B_loc, IN)
        xT = np.empty((IN + 1, L * B_LOC), np.float32)
        xT[:IN] = xc.transpose(2, 0, 1).reshape(IN, L * B_LOC)
        xT[IN] = 1.0
        in_maps.append({"xT": xT, "w_all": w_all, "w_h": w_h, "b_h": b_h})

    res = run_bass_kernel_spmd(nc, in_maps, list(range(NCORES)), trace=_trace)

    mu = np.empty((L, B, OBS), np.float32)
    sg = np.empty((L, B, OBS), np.float32)
    hl = np.empty((1, B, H), np.float32)
    cl = np.empty((1, B, H), np.float32)
    for c in range(NCORES):
        r = res.results[c]
        bs = slice(c * B_LOC, (c + 1) * B_LOC)
        ot = r["outT"].reshape(2 * OBS, L, B_LOC).transpose(1, 2, 0)
        mu[:, bs, :] = ot[:, :, :OBS]
        sg[:, bs, :] = ot[:, :, OBS:]
        hl[0, bs, :] = r["h_last"]
        cl[0, bs, :] = r["c_last"]

    if _trace:
        kernel.last_exec_time_ns = res.exec_time_ns
    return mu, sg, hl, cl


# revision 11
# speedup vs baseline: 1.2035x; 1.0783x over previous
"""DeepAR LSTM forward on 8 Trainium2 NeuronCores.

Strategy (data-parallel over batch):
  - B=1024 split as 128 batch elements per core; the L=512 time recurrence
    runs locally on each core.
  - Cell state is batch-major on chip (batch on partitions, gates on the
    free dim) so every elementwise operand pair shares partition base 0,
    which the walrus verifier requires for SBUF-SBUF TensorTensor ops.
  - ONE matmul per step computes all gate pre-activations: the stationary
    operand is a combined [112, 128] column of a chunk tile holding
    [x_t; 1; 0s; v_{t-1}; 0s; o_{t-1}], the moving operand is a combined
    weight matrix.  The gate bias rides the ones-row.  float32r makes the
    matmul single-pass (vs fp32's LOW+HIGH double pass).
  - tanh is folded into sigmoid: tanh(z) = 2*sigmoid(2z) - 1 (g columns
    pre-scaled by 2), so one sigmoid covers i, f, w=sigma(2g).
  - h = o*tanh(c) is never materialized: with v = sigmoid(2c)*o we have
    h = 2v - o, folded into the recurrent/head weights.  o is transposed
    to feature-major EARLY (right after its sigmoid, off the serial
    chain); after u = sigmoid(2c) is transposed, v is computed directly
    in feature-major form into the next step's stationary slot.
  - Heads are weight-stationary bulk matmuls over 4 steps of (v, o)
    feature-major slots; outputs stay feature-major, host transposes.
"""

import sys
from contextlib import ExitStack

import numpy as np

sys.path.insert(0, "/opt/trn_rl_repo")

import concourse.bacc as bacc  # noqa: E402
import concourse.bass as bass  # noqa: E402
import concourse.mybir as mybir  # noqa: E402
import concourse.tile as tile  # noqa: E402
from concourse.masks import make_identity  # noqa: E402

F32 = mybir.dt.float32
F32R = mybir.dt.float32r
AF = mybir.ActivationFunctionType
ALU = mybir.AluOpType

L, B, IN, H, OBS = 512, 1024, 32, 16, 16
NCORES = 8
B_LOC = B // NCORES          # 128 batch rows per core
T_C = 32                     # timesteps per SBUF chunk
HEAD_G = 4                   # timesteps per head matmul

# gate column order in the [*, 64] gate tensors
GI, GF, GW, GO = 0, 16, 32, 48
# combined stationary rows: x 0:32, ones 32, zeros 33:64, o 64:80,
# zeros 80:96, v 96:112  (o sits at 64 so its transpose-out base is legal)
RV, RO, R1 = 96, 64, 112


def build_nc(steps: int = L, b_loc: int = B_LOC):
    """Emit the per-core Bass program (identical on all cores)."""
    nc = bacc.Bacc(None, target_bir_lowering=False)

    xT = nc.dram_tensor("xT", [IN + 1, steps * b_loc], F32R, kind="ExternalInput")
    w_all = nc.dram_tensor("w_all", [R1, 64], F32R, kind="ExternalInput")
    w_h = nc.dram_tensor("w_h", [48, 2 * OBS], F32R, kind="ExternalInput")
    b_h = nc.dram_tensor("b_h", [2 * OBS, 1], F32, kind="ExternalInput")

    outT = nc.dram_tensor("outT", [2 * OBS, steps * b_loc], F32,
                          kind="ExternalOutput")
    h_last = nc.dram_tensor("h_last", [b_loc, H], F32, kind="ExternalOutput")
    c_last = nc.dram_tensor("c_last", [b_loc, H], F32, kind="ExternalOutput")

    n_chunks = (steps + T_C - 1) // T_C
    assert steps % T_C == 0 and T_C % HEAD_G == 0

    with tile.TileContext(nc) as tc, ExitStack() as ctx:
        singles = ctx.enter_context(tc.tile_pool(name="singles", bufs=1))
        xpool = ctx.enter_context(tc.tile_pool(name="xpool", bufs=3))
        opool = ctx.enter_context(tc.tile_pool(name="opool", bufs=2))
        spool = ctx.enter_context(tc.tile_pool(name="spool", bufs=3))
        small = ctx.enter_context(tc.tile_pool(name="small", bufs=3))
        psum_g = ctx.enter_context(tc.tile_pool(name="psum_g", bufs=2, space="PSUM"))
        psum_u = ctx.enter_context(tc.tile_pool(name="psum_u", bufs=2, space="PSUM"))
        psum_o = ctx.enter_context(tc.tile_pool(name="psum_o", bufs=2, space="PSUM"))
        psum_h = ctx.enter_context(tc.tile_pool(name="psum_h", bufs=2, space="PSUM"))

        # --- constants ---
        wall = singles.tile([R1, 64], F32R)
        # head weights at partitions 64:112 so the head matmul's operands
        # share base partition 64 (hardware requirement); rows 80:96 zero
        wh_t = singles.tile([R1, 2 * OBS], F32R)
        wh = wh_t[RO:R1, :]
        bh = singles.tile([2 * OBS, 1], F32)
        ident = singles.tile([128, 128], F32)
        nc.sync.dma_start(out=wall, in_=w_all[:])
        nc.sync.dma_start(out=wh, in_=w_h[:])
        nc.sync.dma_start(out=bh, in_=b_h[:])
        make_identity(nc, ident)

        cA = singles.tile([b_loc, H], F32)
        cB = singles.tile([b_loc, H], F32)
        nc.vector.memset(cA, 0.0)
        c_tiles = (cA, cB)

        # trailing stationary slot for step L-1's (v, o) -> step-511 heads
        xlast = singles.tile([R1, b_loc], F32R)
        nc.gpsimd.memset(xlast[64:96, :].bitcast(F32), 0.0)

        def new_chunk(ck):
            """Allocate chunk ck's stationary tile, queue DMA + memsets."""
            t0 = ck * T_C
            xc = xpool.tile([R1, T_C * b_loc], F32R)
            # zero bands (DMA rewrites the ones-row 32 afterwards); v slots
            # 64:80 are overwritten per step, 80:96 stays zero for heads
            nc.gpsimd.memset(xc[32:64, :].bitcast(F32), 0.0)
            nc.gpsimd.memset(xc[64:96, :].bitcast(F32), 0.0)
            # v band slot 0 is written by the previous chunk's last step;
            # later slots by this chunk's steps
            nc.sync.dma_start(
                out=xc[: IN + 1, :],
                in_=xT[:, t0 * b_loc : (t0 + T_C) * b_loc],
            )
            if ck == 0:
                nc.gpsimd.memset(xc[RV:R1, :b_loc].bitcast(F32), 0.0)
            return xc

        xc_cur = new_chunk(0)
        xc_next = new_chunk(1) if n_chunks > 1 else None
        last_u = None
        last_ob = None

        for ck in range(n_chunks):
            t0 = ck * T_C
            oc = opool.tile([2 * OBS, T_C * b_loc], F32)

            for tl in range(T_C):
                t = t0 + tl
                sl = slice(tl * b_loc, (tl + 1) * b_loc)
                if tl + 1 < T_C:
                    nsl = slice((tl + 1) * b_loc, (tl + 2) * b_loc)
                    xc_dst = xc_cur
                elif xc_next is not None:
                    nsl = slice(0, b_loc)
                    xc_dst = xc_next
                else:
                    nsl = slice(0, b_loc)
                    xc_dst = xlast

                pg = psum_g.tile([b_loc, 64], F32)
                nc.tensor.matmul(pg, xc_cur[0:R1, sl], wall,
                                 start=True, stop=True)

                s = spool.tile([b_loc, 48], F32)
                ob = small.tile([b_loc, H], F32)
                nc.scalar.activation(out=s, in_=pg[:, 0:48], func=AF.Sigmoid)
                nc.scalar.activation(out=ob, in_=pg[:, 48:64], func=AF.Sigmoid)

                # o -> feature-major early, off the serial chain
                # (transpose outputs must land at PSUM partition 0; the copy
                # shifts to the o band at partition 64)
                po = psum_o.tile([H, b_loc], F32)
                nc.tensor.transpose(po[:], ob[:], ident[:])
                nc.vector.tensor_copy(xc_dst[RO : RO + H, nsl], po[:])

                c_prev = c_tiles[t % 2]
                c_new = c_tiles[(t + 1) % 2]
                wt = small.tile([b_loc, H], F32)
                t1m = small.tile([b_loc, H], F32)
                m1 = small.tile([b_loc, H], F32)
                nc.vector.tensor_scalar(
                    out=wt, in0=s[:, GW : GW + H], scalar1=2.0, scalar2=-1.0,
                    op0=ALU.mult, op1=ALU.add,
                )
                # f*c_prev on GpSimd, off the DVE chain
                nc.gpsimd.tensor_tensor(m1, s[:, GF : GF + H], c_prev,
                                        op=ALU.mult)
                nc.vector.tensor_tensor(t1m, s[:, GI : GI + H], wt, op=ALU.mult)
                nc.vector.tensor_tensor(c_new, m1, t1m, op=ALU.add)

                u = small.tile([b_loc, H], F32)
                nc.scalar.activation(out=u, in_=c_new, func=AF.Sigmoid, scale=2.0)

                # u -> feature-major, then v = u*o directly feature-major
                pu = psum_u.tile([H, b_loc], F32)
                nc.tensor.transpose(pu[:], u[:], ident[:])
                nc.vector.tensor_tensor(
                    xc_dst[RV : RV + H, nsl], pu[:], xc_dst[RO : RO + H, nsl],
                    op=ALU.mult,
                )
                last_u, last_ob = u, ob

                # heads over completed 4-slot windows [4k, 4k+4)
                if tl % HEAD_G == HEAD_G - 2:
                    w0 = tl + 2 - HEAD_G
                    wsl = slice(w0 * b_loc, (w0 + HEAD_G) * b_loc)
                    ph = psum_h.tile([2 * OBS, HEAD_G * b_loc], F32)
                    nc.tensor.matmul(ph, wh, xc_cur[RO:R1, wsl],
                                     start=True, stop=True)
                    nc.scalar.activation(out=oc[:, wsl], in_=ph[:],
                                         func=AF.Identity, bias=bh[:])

            # chunk ck's slots s hold (v,o)_{t0+s-1} -> output steps t0-1..t0+30
            if ck == 0:
                nc.sync.dma_start(
                    out=outT[:, 0 : (T_C - 1) * b_loc],
                    in_=oc[:, b_loc:],
                )
            else:
                nc.sync.dma_start(
                    out=outT[:, (t0 - 1) * b_loc : (t0 + T_C - 1) * b_loc],
                    in_=oc[:],
                )
            xc_cur = xc_next
            xc_next = new_chunk(ck + 2) if ck + 2 < n_chunks else None

        # final step's heads from xlast
        ph = psum_h.tile([2 * OBS, b_loc], F32)
        ol = small.tile([2 * OBS, b_loc], F32)
        nc.tensor.matmul(ph, wh, xlast[RO:R1, :], start=True, stop=True)
        nc.scalar.activation(out=ol, in_=ph[:], func=AF.Identity, bias=bh[:])
        nc.sync.dma_start(out=outT[:, (steps - 1) * b_loc :], in_=ol[:])

        # h = o*(2u - 1) batch-major from the final step's u, o
        th = small.tile([b_loc, H], F32)
        hl = small.tile([b_loc, H], F32)
        nc.vector.tensor_tensor(th, last_u, last_ob, op=ALU.mult)
        nc.vector.scalar_tensor_tensor(
            out=hl, in0=th, scalar=2.0, in1=last_ob,
            op0=ALU.mult, op1=ALU.subtract,
        )
        nc.sync.dma_start(out=h_last[:], in_=hl[:])
        nc.sync.dma_start(out=c_last[:], in_=c_tiles[steps % 2][:])

    return nc


def prep_weights(W_ih, W_hh, b_ih, b_hh, W_mu, b_mu, W_sig, b_sig):
    W = np.asarray(W_ih, np.float32)        # (64, 32)  rows i,f,g,o
    U = np.asarray(W_hh, np.float32)        # (64, 16)
    b = np.asarray(b_ih, np.float32) + np.asarray(b_hh, np.float32)
    Wm = np.asarray(W_mu, np.float32)       # (16, 16)
    Ws = np.asarray(W_sig, np.float32)

    # gate columns [i, f, w(g), o]; g block pre-scaled by 2 for the tanh fold
    w_all = np.zeros((R1, 64), np.float32)
    for bi in range(4):
        r = slice(bi * H, (bi + 1) * H)
        col = slice(bi * H, (bi + 1) * H)
        sc = 2.0 if bi == 2 else 1.0
        w_all[:IN, col] = sc * W[r].T
        w_all[IN, col] = sc * b[r]
        w_all[RV : RV + H, col] = sc * 2.0 * U[r].T
        w_all[RO : RO + H, col] = sc * -1.0 * U[r].T

    # head weight rows match stationary rows 64:112: o, zeros, v
    w_h = np.zeros((48, 2 * OBS), np.float32)
    w_h[0:16, :OBS] = -Wm.T
    w_h[0:16, OBS:] = -Ws.T
    w_h[32:48, :OBS] = 2.0 * Wm.T
    w_h[32:48, OBS:] = 2.0 * Ws.T
    b_h = np.concatenate([np.asarray(b_mu, np.float32),
                          np.asarray(b_sig, np.float32)]).reshape(2 * OBS, 1)
    return w_all, w_h, b_h


def kernel(external_input_seq, W_ih, W_hh, b_ih, b_hh, W_mu, b_mu, W_sig, b_sig,
           _trace=False):
    from concourse.bass_utils import run_bass_kernel_spmd

    x = np.asarray(external_input_seq, np.float32)      # (L, B, IN)
    w_all, w_h, b_h = prep_weights(W_ih, W_hh, b_ih, b_hh, W_mu, b_mu,
                                   W_sig, b_sig)

    nc = build_nc(L, B_LOC)
    nc.compile()
    in_maps = []
    for c in range(NCORES):
        xc = x[:, c * B_LOC : (c + 1) * B_LOC, :]       # (L, B_loc, IN)
        xT = np.empty((IN + 1, L * B_LOC), np.float32)
        xT[:IN] = xc.transpose(2, 0, 1).reshape(IN, L * B_LOC)
        xT[IN] = 1.0
        in_maps.append({"xT": xT, "w_all": w_all, "w_h": w_h, "b_h": b_h})

    res = run_bass_kernel_spmd(nc, in_maps, list(range(NCORES)), trace=_trace)

    mu = np.empty((L, B, OBS), np.float32)
    sg = np.empty((L, B, OBS), np.float32)
    hl = np.empty((1, B, H), np.float32)
    cl = np.empty((1, B, H), np.float32)
    for c in range(NCORES):
        r = res.results[c]
        bs = slice(c * B_LOC, (c + 1) * B_LOC)
        ot = r["outT"].reshape(2 * OBS, L, B_LOC).transpose(1, 2, 0)
        mu[:, bs, :] = ot[:, :, :OBS]
        sg[:, bs, :] = ot[:, :, OBS:]
        hl[0, bs, :] = r["h_last"]
        cl[0, bs, :] = r["c_last"]

    if _trace:
        kernel.last_exec_time_ns = res.exec_time_ns
    return mu, sg, hl, cl


# revision 14
# speedup vs baseline: 1.3152x; 1.0928x over previous
"""DeepAR LSTM forward on 8 Trainium2 NeuronCores.

Strategy (data-parallel over batch):
  - B=1024 split as 128 batch elements per core; the L=512 time recurrence
    runs locally on each core.
  - Cell state is batch-major on chip (batch on partitions, gates on the
    free dim) so every elementwise operand pair shares partition base 0,
    which the walrus verifier requires for SBUF-SBUF TensorTensor ops.
  - ONE matmul per step computes all gate pre-activations: the stationary
    operand is a combined [112, 128] column of a chunk tile holding
    [x_t; 1; 0s; v_{t-1}; 0s; o_{t-1}], the moving operand is a combined
    weight matrix.  The gate bias rides the ones-row.  float32r makes the
    matmul single-pass (vs fp32's LOW+HIGH double pass).
  - tanh is folded into sigmoid: tanh(z) = 2*sigmoid(2z) - 1 (g columns
    pre-scaled by 2), so one sigmoid covers i, f, w=sigma(2g).
  - h = o*tanh(c) is never materialized: with v = sigmoid(2c)*o we have
    h = 2v - o, folded into the recurrent/head weights.  o is transposed
    to feature-major EARLY (right after its sigmoid, off the serial
    chain); after u = sigmoid(2c) is transposed, v is computed directly
    in feature-major form into the next step's stationary slot.
  - Heads are weight-stationary bulk matmuls over 4 steps of (v, o)
    feature-major slots; outputs stay feature-major, host transposes.
"""

import sys
from contextlib import ExitStack

import numpy as np

sys.path.insert(0, "/opt/trn_rl_repo")

import concourse.bacc as bacc  # noqa: E402
import concourse.bass as bass  # noqa: E402
import concourse.mybir as mybir  # noqa: E402
import concourse.tile as tile  # noqa: E402

F32 = mybir.dt.float32
F32R = mybir.dt.float32r
AF = mybir.ActivationFunctionType
ALU = mybir.AluOpType

L, B, IN, H, OBS = 512, 1024, 32, 16, 16
NCORES = 8
B_LOC = B // NCORES          # 128 batch rows per core
T_C = 32                     # timesteps per SBUF chunk
HEAD_G = 4                   # timesteps per head matmul

# gate column order in the [*, 64] gate tensors
GI, GF, GW, GO = 0, 16, 32, 48
# combined stationary rows: x 0:32, ones 32, zeros 33:64, o 64:80,
# zeros 80:96, v 96:112  (o sits at 64 so its transpose-out base is legal)
RV, RO, R1 = 96, 64, 112


def build_nc(steps: int = L, b_loc: int = B_LOC):
    """Emit the per-core Bass program (identical on all cores)."""
    nc = bacc.Bacc(None, target_bir_lowering=False)

    xT = nc.dram_tensor("xT", [IN + 1, steps * b_loc], F32R, kind="ExternalInput")
    w_all = nc.dram_tensor("w_all", [R1, 64], F32R, kind="ExternalInput")
    w_h = nc.dram_tensor("w_h", [48, 2 * OBS], F32R, kind="ExternalInput")
    b_h = nc.dram_tensor("b_h", [2 * OBS, 1], F32, kind="ExternalInput")
    id_in = nc.dram_tensor("id_in", [128, 128], F32R, kind="ExternalInput")

    outT = nc.dram_tensor("outT", [2 * OBS, steps * b_loc], F32,
                          kind="ExternalOutput")
    h_last = nc.dram_tensor("h_last", [b_loc, H], F32, kind="ExternalOutput")
    c_last = nc.dram_tensor("c_last", [b_loc, H], F32, kind="ExternalOutput")

    n_chunks = (steps + T_C - 1) // T_C
    assert steps % T_C == 0 and T_C % HEAD_G == 0

    with tile.TileContext(nc) as tc, ExitStack() as ctx:
        singles = ctx.enter_context(tc.tile_pool(name="singles", bufs=1))
        xpool = ctx.enter_context(tc.tile_pool(name="xpool", bufs=3))
        opool = ctx.enter_context(tc.tile_pool(name="opool", bufs=2))
        spool = ctx.enter_context(tc.tile_pool(name="spool", bufs=3))
        small = ctx.enter_context(tc.tile_pool(name="small", bufs=3))
        psum_g = ctx.enter_context(tc.tile_pool(name="psum_g", bufs=2, space="PSUM"))
        psum_u = ctx.enter_context(tc.tile_pool(name="psum_u", bufs=2, space="PSUM"))
        psum_o = ctx.enter_context(tc.tile_pool(name="psum_o", bufs=2, space="PSUM"))
        psum_h = ctx.enter_context(tc.tile_pool(name="psum_h", bufs=2, space="PSUM"))

        # --- constants ---
        wall = singles.tile([R1, 64], F32R)
        # head weights at partitions 64:112 so the head matmul's operands
        # share base partition 64 (hardware requirement); rows 80:96 zero
        wh_t = singles.tile([R1, 2 * OBS], F32R)
        wh = wh_t[RO:R1, :]
        bh = singles.tile([2 * OBS, 1], F32)
        identr = singles.tile([128, 128], F32R)
        nc.sync.dma_start(out=identr, in_=id_in[:])
        nc.sync.dma_start(out=wall, in_=w_all[:])
        nc.sync.dma_start(out=wh, in_=w_h[:])
        nc.sync.dma_start(out=bh, in_=b_h[:])

        cA = singles.tile([b_loc, H], F32)
        cB = singles.tile([b_loc, H], F32)
        nc.vector.memset(cA, 0.0)
        c_tiles = (cA, cB)

        # trailing stationary slot for step L-1's (v, o) -> step-511 heads
        xlast = singles.tile([R1, b_loc], F32R)
        nc.gpsimd.memset(xlast[64:96, :].bitcast(F32), 0.0)

        def new_chunk(ck):
            """Allocate chunk ck's stationary tile, queue DMA + memsets."""
            t0 = ck * T_C
            xc = xpool.tile([R1, T_C * b_loc], F32R)
            # zero bands (DMA rewrites the ones-row 32 afterwards); v slots
            # 64:80 are overwritten per step, 80:96 stays zero for heads
            nc.gpsimd.memset(xc[32:64, :].bitcast(F32), 0.0)
            nc.gpsimd.memset(xc[64:96, :].bitcast(F32), 0.0)
            # v band slot 0 is written by the previous chunk's last step;
            # later slots by this chunk's steps
            nc.sync.dma_start(
                out=xc[: IN + 1, :],
                in_=xT[:, t0 * b_loc : (t0 + T_C) * b_loc],
            )
            if ck == 0:
                nc.gpsimd.memset(xc[RV:R1, :b_loc].bitcast(F32), 0.0)
            return xc

        xc_cur = new_chunk(0)
        xc_next = new_chunk(1) if n_chunks > 1 else None
        last_u = None
        last_ob = None

        for ck in range(n_chunks):
            t0 = ck * T_C
            oc = opool.tile([2 * OBS, T_C * b_loc], F32)

            for tl in range(T_C):
                t = t0 + tl
                sl = slice(tl * b_loc, (tl + 1) * b_loc)
                if tl + 1 < T_C:
                    nsl = slice((tl + 1) * b_loc, (tl + 2) * b_loc)
                    xc_dst = xc_cur
                elif xc_next is not None:
                    nsl = slice(0, b_loc)
                    xc_dst = xc_next
                else:
                    nsl = slice(0, b_loc)
                    xc_dst = xlast

                pg = psum_g.tile([b_loc, 64], F32)
                nc.tensor.matmul(pg, xc_cur[0:R1, sl], wall,
                                 start=True, stop=True)

                s = spool.tile([b_loc, 48], F32)
                ob = small.tile([b_loc, H], F32R)
                nc.scalar.activation(out=s, in_=pg[:, 0:48], func=AF.Sigmoid)
                nc.scalar.activation(out=ob, in_=pg[:, 48:64], func=AF.Sigmoid)

                c_prev = c_tiles[t % 2]
                c_new = c_tiles[(t + 1) % 2]
                wt = small.tile([b_loc, H], F32)
                t1m = small.tile([b_loc, H], F32)
                m1 = small.tile([b_loc, H], F32)
                nc.vector.tensor_scalar(
                    out=wt, in0=s[:, GW : GW + H], scalar1=2.0, scalar2=-1.0,
                    op0=ALU.mult, op1=ALU.add,
                )
                nc.vector.tensor_tensor(t1m, s[:, GI : GI + H], wt, op=ALU.mult)
                nc.vector.tensor_tensor(m1, s[:, GF : GF + H], c_prev,
                                        op=ALU.mult)
                nc.vector.tensor_tensor(c_new, m1, t1m, op=ALU.add)

                # o -> feature-major, off the serial chain (transpose
                # outputs must land at PSUM base 0; the copy shifts to the
                # o band at partition 64).  Emitted here so the DVE copy
                # queues after c_new and drains during the u sigmoid.
                po = psum_o.tile([H, b_loc], F32R)
                nc.tensor.transpose(po[:], ob[:], identr[:])
                nc.vector.tensor_copy(xc_dst[RO : RO + H, nsl], po[:])

                u = small.tile([b_loc, H], F32R)
                nc.scalar.activation(out=u, in_=c_new, func=AF.Sigmoid, scale=2.0)

                # u -> feature-major, then v = u*o directly feature-major
                pu = psum_u.tile([H, b_loc], F32R)
                nc.tensor.transpose(pu[:], u[:], identr[:])
                nc.vector.tensor_tensor(
                    xc_dst[RV : RV + H, nsl], pu[:], xc_dst[RO : RO + H, nsl],
                    op=ALU.mult,
                )
                last_u, last_ob = u, ob

                # heads over completed 4-slot windows [4k, 4k+4)
                if tl % HEAD_G == HEAD_G - 2:
                    w0 = tl + 2 - HEAD_G
                    wsl = slice(w0 * b_loc, (w0 + HEAD_G) * b_loc)
                    ph = psum_h.tile([2 * OBS, HEAD_G * b_loc], F32)
                    nc.tensor.matmul(ph, wh, xc_cur[RO:R1, wsl],
                                     start=True, stop=True)
                    nc.scalar.activation(out=oc[:, wsl], in_=ph[:],
                                         func=AF.Identity, bias=bh[:])

            # chunk ck's slots s hold (v,o)_{t0+s-1} -> output steps t0-1..t0+30
            if ck == 0:
                nc.sync.dma_start(
                    out=outT[:, 0 : (T_C - 1) * b_loc],
                    in_=oc[:, b_loc:],
                )
            else:
                nc.sync.dma_start(
                    out=outT[:, (t0 - 1) * b_loc : (t0 + T_C - 1) * b_loc],
                    in_=oc[:],
                )
            xc_cur = xc_next
            xc_next = new_chunk(ck + 2) if ck + 2 < n_chunks else None

        # final step's heads from xlast
        ph = psum_h.tile([2 * OBS, b_loc], F32)
        ol = small.tile([2 * OBS, b_loc], F32)
        nc.tensor.matmul(ph, wh, xlast[RO:R1, :], start=True, stop=True)
        nc.scalar.activation(out=ol, in_=ph[:], func=AF.Identity, bias=bh[:])
        nc.sync.dma_start(out=outT[:, (steps - 1) * b_loc :], in_=ol[:])

        # h = o*(2u - 1) batch-major from the final step's u, o
        th = small.tile([b_loc, H], F32)
        hl = small.tile([b_loc, H], F32)
        nc.vector.tensor_tensor(th, last_u, last_ob, op=ALU.mult)
        nc.vector.scalar_tensor_tensor(
            out=hl, in0=th, scalar=2.0, in1=last_ob,
            op0=ALU.mult, op1=ALU.subtract,
        )
        nc.sync.dma_start(out=h_last[:], in_=hl[:])
        nc.sync.dma_start(out=c_last[:], in_=c_tiles[steps % 2][:])

    return nc


def prep_weights(W_ih, W_hh, b_ih, b_hh, W_mu, b_mu, W_sig, b_sig):
    W = np.asarray(W_ih, np.float32)        # (64, 32)  rows i,f,g,o
    U = np.asarray(W_hh, np.float32)        # (64, 16)
    b = np.asarray(b_ih, np.float32) + np.asarray(b_hh, np.float32)
    Wm = np.asarray(W_mu, np.float32)       # (16, 16)
    Ws = np.asarray(W_sig, np.float32)

    # gate columns [i, f, w(g), o]; g block pre-scaled by 2 for the tanh fold
    w_all = np.zeros((R1, 64), np.float32)
    for bi in range(4):
        r = slice(bi * H, (bi + 1) * H)
        col = slice(bi * H, (bi + 1) * H)
        sc = 2.0 if bi == 2 else 1.0
        w_all[:IN, col] = sc * W[r].T
        w_all[IN, col] = sc * b[r]
        w_all[RV : RV + H, col] = sc * 2.0 * U[r].T
        w_all[RO : RO + H, col] = sc * -1.0 * U[r].T

    # head weight rows match stationary rows 64:112: o, zeros, v
    w_h = np.zeros((48, 2 * OBS), np.float32)
    w_h[0:16, :OBS] = -Wm.T
    w_h[0:16, OBS:] = -Ws.T
    w_h[32:48, :OBS] = 2.0 * Wm.T
    w_h[32:48, OBS:] = 2.0 * Ws.T
    b_h = np.concatenate([np.asarray(b_mu, np.float32),
                          np.asarray(b_sig, np.float32)]).reshape(2 * OBS, 1)
    return w_all, w_h, b_h


def kernel(external_input_seq, W_ih, W_hh, b_ih, b_hh, W_mu, b_mu, W_sig, b_sig,
           _trace=False):
    from concourse.bass_utils import run_bass_kernel_spmd

    x = np.asarray(external_input_seq, np.float32)      # (L, B, IN)
    w_all, w_h, b_h = prep_weights(W_ih, W_hh, b_ih, b_hh, W_mu, b_mu,
                                   W_sig, b_sig)

    nc = build_nc(L, B_LOC)
    nc.compile()
    in_maps = []
    for c in range(NCORES):
        xc = x[:, c * B_LOC : (c + 1) * B_LOC, :]       # (L, B_loc, IN)
        xT = np.empty((IN + 1, L * B_LOC), np.float32)
        xT[:IN] = xc.transpose(2, 0, 1).reshape(IN, L * B_LOC)
        xT[IN] = 1.0
        in_maps.append({"xT": xT, "w_all": w_all, "w_h": w_h, "b_h": b_h,
                        "id_in": np.eye(128, dtype=np.float32)})

    res = run_bass_kernel_spmd(nc, in_maps, list(range(NCORES)), trace=_trace)

    mu = np.empty((L, B, OBS), np.float32)
    sg = np.empty((L, B, OBS), np.float32)
    hl = np.empty((1, B, H), np.float32)
    cl = np.empty((1, B, H), np.float32)
    for c in range(NCORES):
        r = res.results[c]
        bs = slice(c * B_LOC, (c + 1) * B_LOC)
        ot = r["outT"].reshape(2 * OBS, L, B_LOC).transpose(1, 2, 0)
        mu[:, bs, :] = ot[:, :, :OBS]
        sg[:, bs, :] = ot[:, :, OBS:]
        hl[0, bs, :] = r["h_last"]
        cl[0, bs, :] = r["c_last"]

    if _trace:
        kernel.last_exec_time_ns = res.exec_time_ns
    return mu, sg, hl, cl
